# revision 13
# baseline (speedup 1.0000x reference)
"""Trainium2 Bass kernel for the Compute_Add_Attention module.

reference math (B=2048, C=32, H=1024):
    proj_c     = cell_state_c @ W_in.T                      [B, 2H]
    proj_cells = einsum('bch,kh->bck', cell_states, W_in)   [B, C, 2H]
    scores     = einsum('bck,k->bc', tanh(proj_c[:,None,:] + proj_cells), v)
    probs      = softmax(scores, axis=1)
    atten      = einsum('bc,bch->bh', probs, cell_states)
    returns (atten, probs)

Both projections share W_in, so
    tanh(proj_c[:,None,:] + proj_cells) = tanh((cell_state_c[:,None,:] + cell_states) @ W_in.T)

Sharding: data-parallel over batch across 8 cores (256 rows each); W_in and
atten_v replicated.

Per-core layout (c-major, transposed):
  z_T[n, (c, b)] = sum_h W[n, h] * s_T[h, (c, b)]   -- PE, lhsT = W_T block
  t = tanh(z_T)                                     -- ScalarE from PSUM
  scores[(c, b)] = sum_n v[n] * t[n, (c, b)]        -- PE, lhsT = v column
  scores -> "fold" layout [b%128, 2c+bhalf] via PE transposes of 128-chunks
  softmax over c in fold layout (exp on ScalarE, strided reduce on DVE)
  atten[b, :] = (sum_c E[c, b] * cells[b, c, :]) / denom[b]
      -- fused DVE scalar_tensor_tensor accumulation, overlapped per pair
"""
import sys

if '/opt/trn_rl_repo' not in sys.path:
    sys.path.insert(0, '/opt/trn_rl_repo')

import numpy as np

import concourse.bacc as bacc
import concourse.bass as bass
import concourse.mybir as mybir
import concourse.tile as tile
from concourse.bass_utils import run_bass_kernel_spmd

F32 = mybir.dt.float32
F32R = mybir.dt.float32r  # fp32 "replicated" matmul mode: 1 col/cycle on PE
                          # (plain fp32 matmul costs 4 cycles/col)

N_CORES = 8
B_FULL = 2048
C = 32
H = 1024


def build_attention_nc(Bs: int, C: int, H: int) -> bass.Bass:
    """Build the single-core Bass program (SPMD: same program on every core)."""
    NH = 2 * H
    KCH = H // 128        # contraction chunks
    NCH = NH // 128       # output-n chunks
    NPAIR = C // 2        # c processed in pairs -> free dim 2*Bs per tile
    NBH = Bs // 128       # batch "halves" of 128
    ROWS = 2 * Bs         # free dim of one pair tile
    assert H % 128 == 0 and C % 2 == 0 and Bs % 128 == 0 and ROWS % 128 == 0

    nc = bacc.Bacc("TRN2", target_bir_lowering=False, debug=False)

    w_t = nc.dram_tensor("w_t", [H, NH], F32R, kind="ExternalInput")
    v_t = nc.dram_tensor("v_t", [128, NCH], F32R, kind="ExternalInput")
    xc2 = nc.dram_tensor("xc2", [128, KCH * ROWS], F32, kind="ExternalInput")
    cells_t = nc.dram_tensor("cells_t", [H, C, Bs], F32, kind="ExternalInput")
    cells_r = nc.dram_tensor("cells_r", [Bs, C, H], F32, kind="ExternalInput")
    atten = nc.dram_tensor("atten", [Bs, H], F32, kind="ExternalOutput")
    probs = nc.dram_tensor("probs", [Bs, C], F32, kind="ExternalOutput")

    TANH = mybir.ActivationFunctionType.Tanh
    EXP = mybir.ActivationFunctionType.Exp
    MULT = mybir.AluOpType.mult
    ADD = mybir.AluOpType.add

    with tile.TileContext(nc) as tc:
        with (
            tc.tile_pool(name="const", bufs=1) as constp,
            tc.tile_pool(name="ctp", bufs=2) as ctp,
            tc.tile_pool(name="sp", bufs=12) as sp,
            tc.tile_pool(name="tp", bufs=4) as tp,
            tc.tile_pool(name="ccp", bufs=3) as ccp,
            tc.tile_pool(name="accp", bufs=NBH) as accp,
            tc.tile_pool(name="miscp", bufs=2) as miscp,
            tc.tile_pool(name="zps", bufs=5, space="PSUM") as zps,
            tc.tile_pool(name="scps", bufs=2, space="PSUM") as scps,
            tc.tile_pool(name="tfps", bufs=1, space="PSUM") as tfps,
        ):
            # ---- resident constants ----
            w_sb = constp.tile([128, KCH * NH], F32R, tag="w_sb")
            nc.gpsimd.dma_start(
                w_sb[:].rearrange("p (k n) -> p k n", k=KCH),
                w_t[:].rearrange("(k p) n -> p k n", p=128),
            )
            v_sb = constp.tile([128, NCH], F32R, tag="v_sb")
            nc.gpsimd.dma_start(v_sb[:], v_t[:])
            xc2_sb = constp.tile([128, KCH * ROWS], F32, tag="xc2_sb")
            nc.gpsimd.dma_start(xc2_sb[:], xc2[:])
            ident = constp.tile([1, 1], F32, tag="ident")
            nc.gpsimd.memset(ident[:], 1.0)

            e_fold = constp.tile([128, C * NBH], F32, tag="e_fold")
            accs = [accp.tile([128, H], F32, tag=f"acc{bh}", name=f"acc{bh}")
                    for bh in range(NBH)]

            # ---- main loop over c pairs ----
            for p in range(NPAIR):
                # cells_t slice for (c=2p, 2p+1): tile free = (k, c2, b)
                ct = ctp.tile([128, KCH * ROWS], F32, tag="ct")
                nc.gpsimd.dma_start(
                    ct[:].rearrange("p (k c b) -> p k c b", k=KCH, c=2),
                    cells_t[:, 2 * p:2 * p + 2, :].rearrange(
                        "(k p) c b -> p k c b", p=128),
                )
                # cells_r slices for step-6 (b-partition layout), per b-half
                ccs = []
                for bh in range(NBH):
                    cc = ccp.tile([128, 2 * H], F32, tag="cc")
                    nc.gpsimd.dma_start(
                        cc[:].rearrange("p (c h) -> p c h", c=2),
                        cells_r[bh * 128:(bh + 1) * 128, 2 * p:2 * p + 2, :],
                    )
                    ccs.append(cc)

                # s_T[k] = cells_T + x_c broadcast
                s_tiles = []
                for k in range(KCH):
                    s = sp.tile([128, ROWS], F32R, tag="s")
                    nc.vector.tensor_add(
                        s[:],
                        ct[:, k * ROWS:(k + 1) * ROWS],
                        xc2_sb[:, k * ROWS:(k + 1) * ROWS],
                    )
                    s_tiles.append(s)

                sc = scps.tile([1, ROWS], F32, tag="sc")
                for nck in range(NCH):
                    z = zps.tile([128, ROWS], F32, tag="z")
                    for k in range(KCH):
                        nc.tensor.matmul(
                            z[:],
                            w_sb[:, k * NH + nck * 128:k * NH + (nck + 1) * 128],
                            s_tiles[k][:],
                            start=(k == 0), stop=(k == KCH - 1),
                        )
                    t = tp.tile([128, ROWS], F32R, tag="t")
                    nc.scalar.activation(t[:], z[:], TANH)
                    nc.tensor.matmul(
                        sc[:], v_sb[:, nck:nck + 1], t[:],
                        start=(nck == 0), stop=(nck == NCH - 1),
                    )

                # scores -> sbuf, transpose 128-chunks into fold layout
                scp = miscp.tile([1, ROWS], F32, tag="scp")
                nc.vector.tensor_copy(scp[:], sc[:])
                nchunk = ROWS // 128   # = 2 * NBH
                tf = tfps.tile([128, nchunk], F32, tag="tf")
                for i in range(nchunk):
                    nc.tensor.transpose(
                        tf[:, i:i + 1], scp[:, i * 128:(i + 1) * 128], ident[:])
                # exp -> e_fold columns (col = 2c + bh, c = 2p + c2)
                nc.scalar.activation(
                    e_fold[:, p * nchunk:(p + 1) * nchunk], tf[:], EXP)

                # step-6 accumulation: acc[bh] += E[c, b] * cells[b, c, :]
                for c2 in range(2):
                    for bh in range(NBH):
                        col = (2 * p + c2) * NBH + bh
                        scal = e_fold[:, col:col + 1]
                        if p == 0 and c2 == 0:
                            nc.vector.tensor_scalar(
                                accs[bh][:], ccs[bh][:, c2 * H:(c2 + 1) * H],
                                scal, None, op0=MULT)
                        else:
                            nc.vector.scalar_tensor_tensor(
                                accs[bh][:], ccs[bh][:, c2 * H:(c2 + 1) * H],
                                scal, accs[bh][:], op0=MULT, op1=ADD)

            # ---- softmax denominators & outputs ----
            denom = miscp.tile([128, NBH], F32, tag="denom")
            nc.vector.tensor_reduce(
                denom[:],
                e_fold[:].rearrange("p (c h) -> p h c", c=C),
                axis=mybir.AxisListType.X, op=ADD)
            recip = miscp.tile([128, NBH], F32, tag="recip")
            nc.vector.reciprocal(recip[:], denom[:])

            probs_fold = miscp.tile([128, C * NBH], F32, tag="probs_fold")
            for bh in range(NBH):
                nc.vector.tensor_scalar(
                    probs_fold[:].rearrange("p (c h) -> p c h", c=C)[:, :, bh],
                    e_fold[:].rearrange("p (c h) -> p c h", c=C)[:, :, bh],
                    recip[:, bh:bh + 1], None, op0=MULT)
                nc.vector.tensor_scalar(
                    accs[bh][:], accs[bh][:], recip[:, bh:bh + 1], None,
                    op0=MULT)

            nc.gpsimd.dma_start(
                probs[:].rearrange("(h p) c -> p c h", p=128),
                probs_fold[:].rearrange("p (c h) -> p c h", c=C),
            )
            for bh in range(NBH):
                nc.gpsimd.dma_start(atten[bh * 128:(bh + 1) * 128, :], accs[bh][:])

    nc.compile()
    return nc


def _prep_core_inputs(cell_state_c, cell_states, W_in, atten_v, core: int,
                      Bs: int):
    """Host-side shard + layout prep for one core. Pure layout, no math."""
    lo, hi = core * Bs, (core + 1) * Bs
    KCH = cell_states.shape[2] // 128
    xc = np.ascontiguousarray(cell_state_c[lo:hi])          # [Bs, H]
    cells = np.ascontiguousarray(cell_states[lo:hi])        # [Bs, C, H]
    # x_c transposed and duplicated along c2: [128, (k, c2, b)]
    xcT = xc.T.reshape(KCH, 128, 1, Bs)                     # [k, p, 1, b]
    xc2 = np.broadcast_to(xcT, (KCH, 128, 2, Bs)).transpose(1, 0, 2, 3)
    xc2 = np.ascontiguousarray(xc2.reshape(128, KCH * 2 * Bs))
    cells_t = np.ascontiguousarray(cells.transpose(2, 1, 0))  # [H, C, Bs]
    return {
        "xc2": xc2,
        "cells_t": cells_t,
        "cells_r": cells,
    }


def kernel(cell_state_c, cell_states, W_in, atten_v):
    cell_state_c = np.asarray(cell_state_c, dtype=np.float32)
    cell_states = np.asarray(cell_states, dtype=np.float32)
    W_in = np.asarray(W_in, dtype=np.float32)
    atten_v = np.asarray(atten_v, dtype=np.float32)

    B, C_, H_ = cell_states.shape
    Bs = B // N_CORES
    NCH = 2 * H_ // 128

    w_t = np.ascontiguousarray(W_in.T)                       # [H, 2H]
    v_t = np.ascontiguousarray(atten_v.reshape(NCH, 128).T)  # [128, NCH]

    nc = build_attention_nc(Bs, C_, H_)

    in_maps = []
    for core in range(N_CORES):
        m = _prep_core_inputs(cell_state_c, cell_states, W_in, atten_v, core, Bs)
        m["w_t"] = w_t
        m["v_t"] = v_t
        in_maps.append(m)

    res = run_bass_kernel_spmd(nc, in_maps, core_ids=list(range(N_CORES)))

    atten = np.concatenate([np.asarray(r["atten"]) for r in res.results], axis=0)
    probs = np.concatenate([np.asarray(r["probs"]) for r in res.results], axis=0)
    return atten.astype(np.float32), probs.astype(np.float32)


if __name__ == "__main__":
    # smoke test with random data at full size
    rng = np.random.default_rng(0)
    inputs = {
        "cell_state_c": rng.standard_normal((B_FULL, H), dtype=np.float32),
        "cell_states": rng.standard_normal((B_FULL, C, H), dtype=np.float32),
        "W_in": rng.standard_normal((2 * H, H), dtype=np.float32) / 32.0,
        "atten_v": rng.standard_normal((2 * H,), dtype=np.float32) / 45.0,
    }
    a, p = kernel(**inputs)
    print("atten", a.shape, "probs", p.shape, p.sum(axis=1)[:4])
